# revision 1
# baseline (speedup 1.0000x reference)
"""Trainium2 Bass kernel for nn_CotLayer (CoT attention layer).

Computation (see reference):
  kemb = relu(grouped_conv3x3(x, Wk, groups=4))
  w1   = relu(We1 @ [x; kemb])            (1x1)
  wbar_k = We2_k @ w1 + be2_k             (per-pixel 3x3 kernel, 8-fold
                                           group replication folded into We2_k)
  xv   = Wv @ x                           (1x1)
  agg  = relu(sum_k shift_k(xv) * wbar_k)
  gap  = mean_{H,W}(agg + kemb)           (AllReduce across 4-core groups)
  attn = sigmoid pair of SE-MLP(gap)
  out  = agg*attn0 + kemb*attn1

Sharding: 8 cores = (batch b) x (H-quarter q); each core computes 64 output
rows; 1-px halo baked into its input slab host-side. x is pre-cast to bf16
on the host; all matmuls are bf16 with fp32 PSUM accumulation.

Engine split: PE does all matmuls + most of the 9-tap aggregation additions
(identity-matmul accumulation in PSUM); DVE does the 9 per-pixel products and
a short bf16 add chain; ACT does PSUM->SBUF relu/copy casts; GPSIMD does the
column-shifted xv copies and the SE row sums.
"""

import numpy as np
import ml_dtypes
from contextlib import ExitStack

import concourse.bass as bass
import concourse.tile as tile
from concourse import bacc, mybir
from concourse.bass_utils import run_bass_kernel_spmd

F32 = mybir.dt.float32
BF16 = mybir.dt.bfloat16
AL = mybir.AluOpType
AF = mybir.ActivationFunctionType
BF = ml_dtypes.bfloat16

B, C, H, W = 2, 128, 256, 256
KSZ, SP = 3, 8
NCORES = 8
RQ = H // 4          # 64 rows per core
TR = 4               # output rows per macro-tile
NT = RQ // TR        # 16 macro-tiles per core
NPX = TR * W         # 1024 px per macro-tile

# ---- tuning knobs ----
GPS_TAPS = ()               # taps whose product-sum chain runs on GPSIMD
WBAR_DVE_TAPS = (1, 4)      # taps whose psum->sbuf copy runs on DVE
XVC_ON_DVE = True    # xv center copy on DVE instead of ACT


def _prep_weights(inputs):
    Wk = np.asarray(inputs["Wk"], np.float32)
    We1 = np.asarray(inputs["We1"], np.float32)[:, :, 0, 0]
    We2 = np.asarray(inputs["We2"], np.float32)[:, :, 0, 0]
    be2 = np.asarray(inputs["be2"], np.float32)
    Wv = np.asarray(inputs["Wv"], np.float32)[:, :, 0, 0]
    Ws1 = np.asarray(inputs["Ws1"], np.float32)[:, :, 0, 0]
    bs1 = np.asarray(inputs["bs1"], np.float32)
    Ws2 = np.asarray(inputs["Ws2"], np.float32)[:, :, 0, 0]
    bs2 = np.asarray(inputs["bs2"], np.float32)

    wk = np.zeros((C, 9, C), np.float32)
    for t in range(9):
        a, b = divmod(t, 3)
        for g in range(4):
            blk = Wk[32 * g:32 * g + 32, :, a, b]
            wk[32 * g:32 * g + 32, t, 32 * g:32 * g + 32] = blk.T
    cidx = (np.arange(C) // SP) * 9
    we2 = np.zeros((64, 9, C), np.float32)
    be2k = np.zeros((C, 9), np.float32)
    for t in range(9):
        we2[:, t, :] = We2[cidx + t, :].T
        be2k[:, t] = be2[cidx + t]
    # taps packed pairwise into disjoint 64-row PE groups: even tap at
    # partitions 0-63, odd tap at 64-127 (reads the duplicated w1 half)
    we2p = np.zeros((C, 5, C), np.float32)
    for jj in range(5):
        we2p[0:64, jj, :] = we2[:, 2 * jj, :]
        if jj < 4:
            we2p[64:C, jj, :] = we2[:, 2 * jj + 1, :]
    ws2 = np.zeros((64, 2, C), np.float32)
    ws2[:, 0, :] = Ws2[0::2, :].T
    ws2[:, 1, :] = Ws2[1::2, :].T
    bs2r = np.zeros((C, 2), np.float32)
    bs2r[:, 0] = bs2[0::2]
    bs2r[:, 1] = bs2[1::2]
    w1x2 = np.concatenate([We1[:, :C].T, We1[:, :C].T], axis=1)   # [128,128]
    w1k2 = np.concatenate([We1[:, C:].T, We1[:, C:].T], axis=1)   # [128,128]
    return dict(
        wk=np.ascontiguousarray(wk.astype(BF)),
        w1x=np.ascontiguousarray(w1x2.astype(BF)),
        w1k=np.ascontiguousarray(w1k2.astype(BF)),
        we2=np.ascontiguousarray(we2p.astype(BF)),
        be2=np.ascontiguousarray(be2k),
        wv=np.ascontiguousarray(Wv.T.astype(BF)),
        ws1=np.ascontiguousarray((Ws1.T / float(H * W)).astype(np.float32)),
        bs1=bs1.reshape(64, 1),
        ws2=np.ascontiguousarray(ws2),
        bs2=bs2r,
        ident=np.ascontiguousarray(np.eye(C, dtype=np.float32).astype(BF)),
    )


def _build_kernel(nc):
    xs = nc.dram_tensor("xs", [C, RQ + 2, W + 2], BF16, kind="ExternalInput")
    wk_d = nc.dram_tensor("wk", [C, 9, C], BF16, kind="ExternalInput")
    w1x_d = nc.dram_tensor("w1x", [C, C], BF16, kind="ExternalInput")
    w1k_d = nc.dram_tensor("w1k", [C, C], BF16, kind="ExternalInput")
    we2_d = nc.dram_tensor("we2", [C, 5, C], BF16, kind="ExternalInput")
    be2_d = nc.dram_tensor("be2", [C, 9], F32, kind="ExternalInput")
    wv_d = nc.dram_tensor("wv", [C, C], BF16, kind="ExternalInput")
    ws1_d = nc.dram_tensor("ws1", [C, 64], F32, kind="ExternalInput")
    bs1_d = nc.dram_tensor("bs1", [64, 1], F32, kind="ExternalInput")
    ws2_d = nc.dram_tensor("ws2", [64, 2, C], F32, kind="ExternalInput")
    bs2_d = nc.dram_tensor("bs2", [C, 2], F32, kind="ExternalInput")
    id_d = nc.dram_tensor("ident", [C, C], BF16, kind="ExternalInput")
    out_d = nc.dram_tensor("out", [C, RQ * W], F32, kind="ExternalOutput")

    cc_in = nc.dram_tensor("cc_in", [C, 1], F32, kind="Internal")
    cc_out = nc.dram_tensor("cc_out", [C, 1], F32, kind="Internal")

    with tile.TileContext(nc) as tc, ExitStack() as ctx:
        singles = ctx.enter_context(tc.tile_pool(name="singles", bufs=1))
        xpool = ctx.enter_context(tc.tile_pool(name="xchunk", bufs=4))
        w1pool = ctx.enter_context(tc.tile_pool(name="w1p", bufs=3))
        wbpool = ctx.enter_context(tc.tile_pool(name="wbp", bufs=2))
        xvpool = ctx.enter_context(tc.tile_pool(name="xvp", bufs=2))
        prodp = ctx.enter_context(tc.tile_pool(name="prodp", bufs=4))
        accp = ctx.enter_context(tc.tile_pool(name="accp", bufs=6))
        p2pool = ctx.enter_context(tc.tile_pool(name="p2p", bufs=2))
        outp = ctx.enter_context(tc.tile_pool(name="outp", bufs=3))
        smallp = ctx.enter_context(tc.tile_pool(name="smallp", bufs=1))
        # PSUM: pbig 2 slots x [128,1024] (kemb, w1, agg rotate) = 4 banks;
        #       pwb 2 slots x [128,1024] (wbar taps, xv pairs) = 4 banks.
        pbig = ctx.enter_context(tc.tile_pool(name="pbig", bufs=2, space="PSUM"))
        pwb = ctx.enter_context(tc.tile_pool(name="pwb", bufs=2, space="PSUM"))

        def sb(name, shape, dt, dram):
            t_ = singles.tile(shape, dt, tag=name)
            nc.sync.dma_start(t_, dram.ap())
            return t_

        wk_sb = sb("wk", [C, 9, C], BF16, wk_d)
        w1x_sb = sb("w1x", [C, C], BF16, w1x_d)
        w1k_sb = sb("w1k", [C, C], BF16, w1k_d)
        we2_sb = sb("we2", [C, 5, C], BF16, we2_d)
        be2_sb = sb("be2", [C, 9], F32, be2_d)
        wv_sb = sb("wv", [C, C], BF16, wv_d)
        ws1_sb = sb("ws1", [C, 64], F32, ws1_d)
        bs1_sb = sb("bs1", [64, 1], F32, bs1_d)
        ws2_sb = sb("ws2", [64, 2, C], F32, ws2_d)
        bs2_sb = sb("bs2", [C, 2], F32, bs2_d)
        id_sb = sb("ident", [C, C], BF16, id_d)

        kemb_slab = singles.tile([C, RQ * W], BF16)
        agg_slab = singles.tile([C, RQ * W], BF16)
        slots_k = singles.tile([C, 2 * NT], F32)
        slots_a = singles.tile([C, NT], F32)
        attn_sb = singles.tile([C, 2], F32)

        # pre-warm the sigmoid ACT table so the SE tail doesn't pay the
        # ~2.7us table load on the critical path
        warm = smallp.tile([C, 1], F32, tag="warm")
        nc.vector.memset(warm, 0.0)
        nc.scalar.activation(warm, warm, AF.Sigmoid)

        # ---------------- phase 1 ----------------
        for t in range(NT):
            xc = xpool.tile([C, TR + 2, W + 2], BF16, tag="xc")
            nc.sync.dma_start(xc, xs.ap()[:, TR * t:TR * t + TR + 2, :])

            # kemb: grouped 3x3 conv as block-diag matmuls; half-granular
            # relu so the w1 matmuls can start on half A early
            pk = pbig.tile([C, NPX], F32, tag="big")
            kv = kemb_slab[:, t * NPX:(t + 1) * NPX]
            for g2 in range(2):
                for tap in range(9):
                    a, b = divmod(tap, 3)
                    nc.tensor.matmul(
                        pk[:, g2 * 512:(g2 + 1) * 512],
                        lhsT=wk_sb[:, tap, :],
                        rhs=xc[:, 2 * g2 + a:2 * g2 + a + 2, b:b + W],
                        start=(tap == 0), stop=(tap == 8),
                    )
                nc.scalar.activation(kv[:, g2 * 512:(g2 + 1) * 512],
                                     pk[:, g2 * 512:(g2 + 1) * 512], AF.Relu,
                                     accum_out=slots_k[:, 2 * t + g2:
                                                       2 * t + g2 + 1])

            # w1 = relu(We1 @ [x; kemb]), duplicated into both 64-row halves
            pw = pbig.tile([C, NPX], F32, tag="big")
            nc.tensor.matmul(pw[:, 0:512], lhsT=w1x_sb,
                             rhs=xc[:, 1:3, 1:1 + W], start=True, stop=False)
            nc.tensor.matmul(pw[:, 512:1024], lhsT=w1x_sb,
                             rhs=xc[:, 3:5, 1:1 + W], start=True, stop=False)
            nc.tensor.matmul(pw[:, 0:512], lhsT=w1k_sb, rhs=kv[:, 0:512],
                             start=False, stop=True)
            nc.tensor.matmul(pw[:, 512:1024], lhsT=w1k_sb, rhs=kv[:, 512:1024],
                             start=False, stop=True)
            w1b = w1pool.tile([C, NPX], BF16, tag="w1")
            nc.scalar.activation(w1b, pw, AF.Relu)

            # xv = Wv @ x over 6 rows; center copy + 2 col-shifted copies
            xvc = xvpool.tile([C, TR + 2, W], BF16, tag="xvc")
            xvl = xvpool.tile([C, TR + 2, W], BF16, tag="xvl")
            xvr = xvpool.tile([C, TR + 2, W], BF16, tag="xvr")
            for m in range(3):
                pxv = pwb.tile([C, 512], F32, tag="wb")
                nc.tensor.matmul(pxv, lhsT=wv_sb,
                                 rhs=xc[:, 2 * m:2 * m + 2, 1:1 + W],
                                 start=True, stop=True)
                pv = pxv.rearrange("p (r w) -> p r w", w=W)
                if XVC_ON_DVE:
                    nc.vector.tensor_scalar(xvc[:, 2 * m:2 * m + 2, :], pv,
                                            0.0, None, AL.add)
                else:
                    nc.scalar.activation(xvc[:, 2 * m:2 * m + 2, :], pv, AF.Copy)
                nc.scalar.activation(xvl[:, 2 * m:2 * m + 2, 0:W - 1],
                                     pv[:, :, 1:W], AF.Copy)
                nc.scalar.activation(xvr[:, 2 * m:2 * m + 2, 1:W],
                                     pv[:, :, 0:W - 1], AF.Copy)
            nc.gpsimd.memset(xvl[:, :, W - 1:W], 0.0)
            nc.gpsimd.memset(xvr[:, :, 0:1], 0.0)

            # wbar_k = We2_k @ w1 + be2_k; taps paired into disjoint
            # 64-row PE groups so the pair's matmuls run concurrently
            wbs = [None] * 9
            for jj in range(5):
                taps = [2 * jj] + ([2 * jj + 1] if jj < 4 else [])
                pbs = {}
                for h in range(2):
                    cs = slice(512 * h, 512 * h + 512)
                    for ti, tap in enumerate(taps):
                        if tap not in pbs:
                            pbt = pwb.tile([C, NPX], F32, tag="wb")
                            pbs[tap] = pbt
                        lo = 64 * ti
                        nc.tensor.matmul(
                            pbs[tap][:, cs],
                            lhsT=we2_sb[lo:lo + 64, jj, :],
                            rhs=w1b[lo:lo + 64, cs],
                            start=True, stop=True)
                for tap in taps:
                    wb = wbpool.tile([C, NPX], BF16, tag=f"wb{tap}")
                    if tap in WBAR_DVE_TAPS:
                        nc.vector.tensor_scalar(wb, pbs[tap],
                                                be2_sb[:, tap:tap + 1],
                                                None, AL.add)
                    else:
                        nc.scalar.activation(wb, pbs[tap], AF.Identity,
                                             bias=be2_sb[:, tap:tap + 1])
                    wbs[tap] = wb

            # aggregation: products on DVE as taps complete; tree-shaped adds
            # so partial sums start as early as possible
            srcs = {0: xvr, 1: xvc, 2: xvl}
            sums = []
            pair = []
            for tap in range(9):
                a, b = divmod(tap, 3)
                xview = srcs[b][:, a:a + TR, :]
                p = prodp.tile([C, NPX], BF16, tag="prod")
                nc.vector.tensor_tensor(p, xview, wbs[tap], AL.mult)
                pair.append(p)
                if len(pair) == 2:
                    s = accp.tile([C, NPX], BF16, tag="acc")
                    nc.vector.tensor_tensor(s, pair[0], pair[1], AL.add)
                    pair = []
                    sums.append(s)
            sums.extend(pair)   # leftover p8
            while len(sums) > 1:
                s = accp.tile([C, NPX], BF16, tag="acc")
                nc.vector.tensor_tensor(s, sums[0], sums[1], AL.add)
                sums = sums[2:] + [s]
            acc = sums[0]
            av = agg_slab[:, t * NPX:(t + 1) * NPX]
            nc.scalar.activation(av, acc, AF.Relu,
                                 accum_out=slots_a[:, t:t + 1])

        # ---------------- SE attention (tiny) ----------------
        sum_k = smallp.tile([C, 1], F32, tag="sk")
        sum_a = smallp.tile([C, 1], F32, tag="sa")
        nc.vector.tensor_reduce(sum_k, slots_k, mybir.AxisListType.X, AL.add)
        nc.vector.tensor_reduce(sum_a, slots_a, mybir.AxisListType.X, AL.add)
        gap = smallp.tile([C, 1], F32, tag="gap")
        nc.vector.tensor_tensor(gap, sum_k, sum_a, AL.add)
        nc.gpsimd.dma_start(cc_in.ap(), gap)
        nc.gpsimd.collective_compute(
            "AllReduce", AL.add,
            replica_groups=[[0, 1, 2, 3], [4, 5, 6, 7]],
            ins=[cc_in.ap().opt()],
            outs=[cc_out.ap().opt()],
        )
        gap2 = smallp.tile([C, 1], F32, tag="gap2")
        nc.gpsimd.dma_start(gap2, cc_out.ap())

        ph = pbig.tile([64, 1], F32, tag="big")
        nc.tensor.matmul(ph, lhsT=ws1_sb, rhs=gap2, start=True, stop=True)
        hso = smallp.tile([64, 1], F32, tag="h")
        nc.scalar.activation(hso, ph, AF.Relu, bias=bs1_sb[:, 0:1])
        pa = pbig.tile([C, 2], F32, tag="big")
        nc.tensor.matmul(pa[:, 0:1], lhsT=ws2_sb[:, 0, :], rhs=hso,
                         start=True, stop=True)
        nc.tensor.matmul(pa[:, 1:2], lhsT=ws2_sb[:, 1, :], rhs=hso,
                         start=True, stop=True)
        a01 = smallp.tile([C, 2], F32, tag="a01")
        nc.scalar.activation(a01[:, 0:1], pa[:, 0:1], AF.Identity,
                             bias=bs2_sb[:, 0:1])
        nc.scalar.activation(a01[:, 1:2], pa[:, 1:2], AF.Identity,
                             bias=bs2_sb[:, 1:2])
        dse = smallp.tile([C, 1], F32, tag="dse")
        nc.vector.tensor_tensor(dse, a01[:, 0:1], a01[:, 1:2], AL.subtract)
        nc.scalar.activation(attn_sb[:, 0:1], dse, AF.Sigmoid)
        nc.scalar.activation(attn_sb[:, 1:2], dse, AF.Sigmoid, scale=-1.0)

        # ---------------- phase 2: blend + store ----------------
        for t in range(NT):
            kv = kemb_slab[:, t * NPX:(t + 1) * NPX]
            av = agg_slab[:, t * NPX:(t + 1) * NPX]
            t1 = p2pool.tile([C, NPX], BF16, tag="t1")
            nc.vector.tensor_scalar(t1, kv, attn_sb[:, 1:2], None, AL.mult)
            outf = outp.tile([C, NPX], F32, tag="outf")
            nc.vector.scalar_tensor_tensor(outf, av, attn_sb[:, 0:1], t1,
                                           AL.mult, AL.add)
            nc.sync.dma_start(out_d.ap()[:, t * NPX:(t + 1) * NPX], outf)

    return nc


_CACHE = {}


def _get_nc():
    if "nc" not in _CACHE:
        nc = bacc.Bacc("TRN2", target_bir_lowering=False, debug=False,
                       num_devices=NCORES)
        _build_kernel(nc)
        nc.compile()
        _CACHE["nc"] = nc
    return _CACHE["nc"]


def make_in_maps(inputs):
    x = np.asarray(inputs["x"], np.float32)
    wts = _prep_weights(inputs)
    xp = np.pad(x, ((0, 0), (0, 0), (1, 1), (1, 1))).astype(BF)
    in_maps = []
    for core in range(NCORES):
        bb, q = divmod(core, 4)
        slab = np.ascontiguousarray(xp[bb, :, RQ * q:RQ * q + RQ + 2, :])
        m = {"xs": slab}
        m.update(wts)
        in_maps.append(m)
    return in_maps


def kernel(**inputs):
    in_maps = make_in_maps(inputs)
    nc = _get_nc()
    res = run_bass_kernel_spmd(nc, in_maps, core_ids=list(range(NCORES)))
    out = np.empty((B, C, H, W), np.float32)
    for core in range(NCORES):
        bb, q = divmod(core, 4)
        out[bb, :, RQ * q:RQ * q + RQ, :] = \
            res.results[core]["out"].reshape(C, RQ, W)
    return out



# revision 3
# speedup vs baseline: 1.3368x; 1.3368x over previous
"""Trainium2 Bass kernel for nn_CotLayer (CoT attention layer).

Computation (see reference):
  kemb = relu(grouped_conv3x3(x, Wk, groups=4))
  w1   = relu(We1 @ [x; kemb])            (1x1)
  wbar_k = We2_k @ w1 + be2_k             (per-pixel 3x3 kernel, 8-fold
                                           group replication folded into We2_k)
  xv   = Wv @ x                           (1x1)
  agg  = relu(sum_k shift_k(xv) * wbar_k)
  gap  = mean_{H,W}(agg + kemb)           (AllReduce across 4-core groups)
  attn = sigmoid pair of SE-MLP(gap)
  out  = agg*attn0 + kemb*attn1

Sharding: 8 cores = (batch b) x (H-quarter q); each core computes 64 output
rows; 1-px halo baked into its input slab host-side. x is pre-cast to bf16
on the host; all matmuls are bf16 with fp32 PSUM accumulation.

v2 design:
 - xv computed once into a persistent column-padded SBUF slab [C, 66, 258]
   (zero pad cols give exact zero-pad conv semantics); per-tap shifted reads
   are free strided views, no shifted copies.
 - wbar taps split: side-column taps get an ACT psum->sbuf bias-copy then a
   2x bf16 DVE product; center-column taps (misaligned views, 1x anyway) use
   a fused scalar_tensor_tensor product straight out of PSUM (bias folded).
 - aggregation: part of the 9-product sum runs as identity-matmul PSUM
   accumulation on PE; the rest is a small DVE add tree.
 - SE global-average-pool AllReduce is split: partial over tiles 0..11 is
   issued early (hidden under tiles 12-15), remainder after tile 15.
 - phase 2 blend out = attn0*agg + attn1*kemb runs on PE as two diagonal
   matmuls per half-tile accumulated in PSUM; psum->dram conversions
   alternate between ACT and DVE.
"""

import numpy as np
import ml_dtypes
from contextlib import ExitStack

import concourse.bass as bass
import concourse.tile as tile
from concourse import bacc, mybir
from concourse.bass_utils import run_bass_kernel_spmd

F32 = mybir.dt.float32
BF16 = mybir.dt.bfloat16
AL = mybir.AluOpType
AF = mybir.ActivationFunctionType
BF = ml_dtypes.bfloat16

B, C, H, W = 2, 128, 256, 256
KSZ, SP = 3, 8
NCORES = 8
RQ = H // 4          # 64 rows per core
TR = 4               # output rows per macro-tile
NT = RQ // TR        # 16 macro-tiles per core
NPX = TR * W         # 1024 px per macro-tile

# ---- tuning knobs ----
PSUM_TAPS = (1, 4, 7)       # taps consumed straight from PSUM via fused STT
PE_ACC_TAPS = (1, 4, 7, 8)  # taps accumulated by PE identity-matmul (+ DVE partial)
SPLIT_T = 12                # tiles in the first (hidden) AllReduce chunk
XV_CONV_DVE = 1             # xv psum->slab conversion: every Nth tile on DVE (0=ACT only)
P2_DVE_MOD = 2              # phase2 conversions: every Nth tile on DVE, rest ACT


def _prep_weights(inputs):
    Wk = np.asarray(inputs["Wk"], np.float32)
    We1 = np.asarray(inputs["We1"], np.float32)[:, :, 0, 0]
    We2 = np.asarray(inputs["We2"], np.float32)[:, :, 0, 0]
    be2 = np.asarray(inputs["be2"], np.float32)
    Wv = np.asarray(inputs["Wv"], np.float32)[:, :, 0, 0]
    Ws1 = np.asarray(inputs["Ws1"], np.float32)[:, :, 0, 0]
    bs1 = np.asarray(inputs["bs1"], np.float32)
    Ws2 = np.asarray(inputs["Ws2"], np.float32)[:, :, 0, 0]
    bs2 = np.asarray(inputs["bs2"], np.float32)

    wk = np.zeros((C, 9, C), np.float32)
    for t in range(9):
        a, b = divmod(t, 3)
        for g in range(4):
            blk = Wk[32 * g:32 * g + 32, :, a, b]
            wk[32 * g:32 * g + 32, t, 32 * g:32 * g + 32] = blk.T
    cidx = (np.arange(C) // SP) * 9
    we2 = np.zeros((64, 9, C), np.float32)
    be2k = np.zeros((C, 9), np.float32)
    for t in range(9):
        we2[:, t, :] = We2[cidx + t, :].T
        be2k[:, t] = be2[cidx + t]
    # taps packed pairwise into disjoint 64-row PE groups: even tap at
    # partitions 0-63, odd tap at 64-127 (reads the duplicated w1 half)
    we2p = np.zeros((C, 5, C), np.float32)
    for jj in range(5):
        we2p[0:64, jj, :] = we2[:, 2 * jj, :]
        if jj < 4:
            we2p[64:C, jj, :] = we2[:, 2 * jj + 1, :]
    ws2 = np.zeros((64, 2, C), np.float32)
    ws2[:, 0, :] = Ws2[0::2, :].T
    ws2[:, 1, :] = Ws2[1::2, :].T
    bs2r = np.zeros((C, 2), np.float32)
    bs2r[:, 0] = bs2[0::2]
    bs2r[:, 1] = bs2[1::2]
    w1x2 = np.concatenate([We1[:, :C].T, We1[:, :C].T], axis=1)   # [128,128]
    w1k2 = np.concatenate([We1[:, C:].T, We1[:, C:].T], axis=1)   # [128,128]
    return dict(
        wk=np.ascontiguousarray(wk.astype(BF)),
        w1x=np.ascontiguousarray(w1x2.astype(BF)),
        w1k=np.ascontiguousarray(w1k2.astype(BF)),
        we2=np.ascontiguousarray(we2p.astype(BF)),
        be2=np.ascontiguousarray(be2k),
        wv=np.ascontiguousarray(Wv.T.astype(BF)),
        ws1=np.ascontiguousarray((Ws1.T / float(H * W)).astype(np.float32)),
        bs1=bs1.reshape(64, 1),
        ws2=np.ascontiguousarray(ws2),
        bs2=bs2r,
        ident=np.ascontiguousarray(np.eye(C, dtype=np.float32).astype(BF)),
    )


def _build_kernel(nc):
    xs = nc.dram_tensor("xs", [C, RQ + 2, W + 2], BF16, kind="ExternalInput")
    wk_d = nc.dram_tensor("wk", [C, 9, C], BF16, kind="ExternalInput")
    w1x_d = nc.dram_tensor("w1x", [C, C], BF16, kind="ExternalInput")
    w1k_d = nc.dram_tensor("w1k", [C, C], BF16, kind="ExternalInput")
    we2_d = nc.dram_tensor("we2", [C, 5, C], BF16, kind="ExternalInput")
    be2_d = nc.dram_tensor("be2", [C, 9], F32, kind="ExternalInput")
    wv_d = nc.dram_tensor("wv", [C, C], BF16, kind="ExternalInput")
    ws1_d = nc.dram_tensor("ws1", [C, 64], F32, kind="ExternalInput")
    bs1_d = nc.dram_tensor("bs1", [64, 1], F32, kind="ExternalInput")
    ws2_d = nc.dram_tensor("ws2", [64, 2, C], F32, kind="ExternalInput")
    bs2_d = nc.dram_tensor("bs2", [C, 2], F32, kind="ExternalInput")
    id_d = nc.dram_tensor("ident", [C, C], BF16, kind="ExternalInput")
    out_d = nc.dram_tensor("out", [C, RQ * W], F32, kind="ExternalOutput")

    cc_inA = nc.dram_tensor("cc_inA", [C, 1], F32, kind="Internal")
    cc_outA = nc.dram_tensor("cc_outA", [C, 1], F32, kind="Internal")
    cc_inB = nc.dram_tensor("cc_inB", [C, 1], F32, kind="Internal")
    cc_outB = nc.dram_tensor("cc_outB", [C, 1], F32, kind="Internal")

    with tile.TileContext(nc) as tc, ExitStack() as ctx:
        singles = ctx.enter_context(tc.tile_pool(name="singles", bufs=1))
        xpool = ctx.enter_context(tc.tile_pool(name="xchunk", bufs=4))
        w1pool = ctx.enter_context(tc.tile_pool(name="w1p", bufs=3))
        wbpool = ctx.enter_context(tc.tile_pool(name="wbp", bufs=4))
        prodp = ctx.enter_context(tc.tile_pool(name="prodp", bufs=12))
        accp = ctx.enter_context(tc.tile_pool(name="accp", bufs=4))
        outp = ctx.enter_context(tc.tile_pool(name="outp", bufs=4))
        smallp = ctx.enter_context(tc.tile_pool(name="smallp", bufs=1))
        # PSUM: poolA (xv chunk / kemb / w1 / SE / phase2) 2x[128,1024] = 4 banks
        #       poolB (wbar taps + agg accumulation)       2x[128,1024] = 4 banks
        poolA = ctx.enter_context(tc.tile_pool(name="poolA", bufs=2, space="PSUM"))
        poolB = ctx.enter_context(tc.tile_pool(name="poolB", bufs=2, space="PSUM"))

        def sb(name, shape, dt, dram):
            t_ = singles.tile(shape, dt, tag=name)
            nc.sync.dma_start(t_, dram.ap())
            return t_

        wk_sb = sb("wk", [C, 9, C], BF16, wk_d)
        w1x_sb = sb("w1x", [C, C], BF16, w1x_d)
        w1k_sb = sb("w1k", [C, C], BF16, w1k_d)
        we2_sb = sb("we2", [C, 5, C], BF16, we2_d)
        be2_sb = sb("be2", [C, 9], F32, be2_d)
        wv_sb = sb("wv", [C, C], BF16, wv_d)
        ws1_sb = sb("ws1", [C, 64], F32, ws1_d)
        bs1_sb = sb("bs1", [64, 1], F32, bs1_d)
        ws2_sb = sb("ws2", [64, 2, C], F32, ws2_d)
        bs2_sb = sb("bs2", [C, 2], F32, bs2_d)
        id_sb = sb("ident", [C, C], BF16, id_d)

        kemb_slab = singles.tile([C, RQ * W], BF16)
        agg_slab = singles.tile([C, RQ * W], BF16)
        # xv slab rows = xv rows -1..64 (slab row i = xv row i-1); cols 0 and
        # 257 are permanent zero pads giving exact zero-pad tap views.
        xv_slab = singles.tile([C, RQ + 2, W + 2], BF16)
        slots_k = singles.tile([C, NT], F32)
        slots_a = singles.tile([C, NT], F32)
        attn_sb = singles.tile([C, 2], F32)
        diag0_sb = singles.tile([C, C], BF16)
        diag1_sb = singles.tile([C, C], BF16)

        nc.gpsimd.memset(xv_slab[:, :, 0:1], 0.0)
        nc.gpsimd.memset(xv_slab[:, :, W + 1:W + 2], 0.0)

        # pre-warm the sigmoid ACT table so the SE tail doesn't pay the
        # ~2.7us table load on the critical path
        warm = smallp.tile([C, 1], F32, tag="warm")
        nc.vector.memset(warm, 0.0)
        nc.scalar.activation(warm, warm, AF.Sigmoid)

        xcs = {}

        def dma_xc(t):
            xc = xpool.tile([C, TR + 2, W + 2], BF16, tag="xc")
            nc.sync.dma_start(xc, xs.ap()[:, TR * t:TR * t + TR + 2, :])
            xcs[t] = xc

        def xv_chunk(c):
            # chunk c: xv rows 4c-1..4c+2 -> slab rows 4c..4c+3, from xc(c)
            xc = xcs[c]
            pxv = poolA.tile([C, NPX], F32, tag="pA")
            nc.tensor.matmul(pxv[:, 0:512], lhsT=wv_sb,
                             rhs=xc[:, 0:2, 1:1 + W], start=True, stop=True)
            nc.tensor.matmul(pxv[:, 512:1024], lhsT=wv_sb,
                             rhs=xc[:, 2:4, 1:1 + W], start=True, stop=True)
            pv = pxv.rearrange("p (r w) -> p r w", w=W)
            dst = xv_slab[:, 4 * c:4 * c + 4, 1:1 + W]
            if XV_CONV_DVE and (c % XV_CONV_DVE == 0):
                nc.vector.tensor_scalar(dst, pv, 0.0, None, AL.add)
            else:
                nc.scalar.activation(dst, pv, AF.Copy)

        # prologue: first input chunk + first xv chunk
        dma_xc(0)
        xv_chunk(0)

        def emit_se(tag, lo, hi, cc_in, cc_out):
            rk = smallp.tile([C, 1], F32, tag=f"rk{tag}")
            ra = smallp.tile([C, 1], F32, tag=f"ra{tag}")
            nc.vector.tensor_reduce(rk, slots_k[:, lo:hi],
                                    mybir.AxisListType.X, AL.add)
            nc.vector.tensor_reduce(ra, slots_a[:, lo:hi],
                                    mybir.AxisListType.X, AL.add)
            gap = smallp.tile([C, 1], F32, tag=f"gap{tag}")
            nc.vector.tensor_tensor(gap, rk, ra, AL.add)
            nc.gpsimd.dma_start(cc_in.ap(), gap)
            nc.gpsimd.collective_compute(
                "AllReduce", AL.add,
                replica_groups=[[0, 1, 2, 3], [4, 5, 6, 7]],
                ins=[cc_in.ap().opt()],
                outs=[cc_out.ap().opt()],
            )
            g2 = smallp.tile([C, 1], F32, tag=f"g2{tag}")
            nc.gpsimd.dma_start(g2, cc_out.ap())
            return g2

        g2A = None

        # ---------------- phase 1 ----------------
        for t in range(NT):
            # prefetch next input tile + compute xv chunk t+1 (tile t's
            # products need xv slab rows up to 4t+5 = chunk t+1)
            if t + 1 < NT:
                dma_xc(t + 1)
                xv_chunk(t + 1)
            else:
                # epilogue chunk: xv rows 63..64 -> slab rows 64..65
                xc15 = xcs[NT - 1]
                pxe = poolA.tile([C, NPX], F32, tag="pA")
                nc.tensor.matmul(pxe[:, 0:512], lhsT=wv_sb,
                                 rhs=xc15[:, 4:6, 1:1 + W], start=True, stop=True)
                pve = pxe[:, 0:512].rearrange("p (r w) -> p r w", w=W)
                nc.scalar.activation(xv_slab[:, RQ:RQ + 2, 1:1 + W], pve, AF.Copy)

            xc = xcs[t]

            # kemb: grouped 3x3 conv as block-diag matmuls, tap-major
            pk = poolA.tile([C, NPX], F32, tag="pA")
            kv = kemb_slab[:, t * NPX:(t + 1) * NPX]
            for tap in range(9):
                a, b = divmod(tap, 3)
                nc.tensor.matmul(pk[:, 0:512], lhsT=wk_sb[:, tap, :],
                                 rhs=xc[:, a:a + 2, b:b + W],
                                 start=(tap == 0), stop=(tap == 8))
                nc.tensor.matmul(pk[:, 512:1024], lhsT=wk_sb[:, tap, :],
                                 rhs=xc[:, a + 2:a + 4, b:b + W],
                                 start=(tap == 0), stop=(tap == 8))
            nc.scalar.activation(kv, pk, AF.Relu,
                                 accum_out=slots_k[:, t:t + 1])

            # w1 = relu(We1 @ [x; kemb]), duplicated into both 64-row halves
            pw = poolA.tile([C, NPX], F32, tag="pA")
            nc.tensor.matmul(pw[:, 0:512], lhsT=w1x_sb,
                             rhs=xc[:, 1:3, 1:1 + W], start=True, stop=False)
            nc.tensor.matmul(pw[:, 512:1024], lhsT=w1x_sb,
                             rhs=xc[:, 3:5, 1:1 + W], start=True, stop=False)
            nc.tensor.matmul(pw[:, 0:512], lhsT=w1k_sb, rhs=kv[:, 0:512],
                             start=False, stop=True)
            nc.tensor.matmul(pw[:, 512:1024], lhsT=w1k_sb, rhs=kv[:, 512:1024],
                             start=False, stop=True)
            w1b = w1pool.tile([C, NPX], BF16, tag="w1")
            nc.scalar.activation(w1b, pw, AF.Relu)

            # wbar taps (paired into disjoint 64-row PE groups) + products
            def xv_view(tap):
                a, b = divmod(tap, 3)
                return xv_slab[:, 4 * t + a:4 * t + a + 4, b:b + W]

            prods = [None] * 9
            for jj in range(5):
                taps = [2 * jj] + ([2 * jj + 1] if jj < 4 else [])
                pbs = {}
                for ti, tap in enumerate(taps):
                    pbs[tap] = poolB.tile([C, NPX], F32, tag="pB",
                                          name=f"pb{tap}")
                for h in range(2):
                    cs = slice(512 * h, 512 * h + 512)
                    for ti, tap in enumerate(taps):
                        lo = 64 * ti
                        nc.tensor.matmul(
                            pbs[tap][:, cs],
                            lhsT=we2_sb[lo:lo + 64, jj, :],
                            rhs=w1b[lo:lo + 64, cs],
                            start=True, stop=True)
                for tap in taps:
                    p = prodp.tile([C, NPX], BF16, tag="prod")
                    if tap in PSUM_TAPS:
                        nc.vector.scalar_tensor_tensor(
                            p, pbs[tap], be2_sb[:, tap:tap + 1], xv_view(tap),
                            AL.add, AL.mult)
                    else:
                        wb = wbpool.tile([C, NPX], BF16, tag="wb")
                        nc.scalar.activation(wb, pbs[tap], AF.Identity,
                                             bias=be2_sb[:, tap:tap + 1])
                        nc.vector.tensor_tensor(p, wb, xv_view(tap), AL.mult)
                    prods[tap] = p

            # aggregation: DVE partial tree over non-PE taps, then PE
            # identity-matmul accumulation of the rest + partial into PSUM
            dve_taps = [k for k in range(9) if k not in PE_ACC_TAPS]
            sums = [prods[k] for k in dve_taps]
            while len(sums) > 1:
                s = accp.tile([C, NPX], BF16, tag="acc")
                nc.vector.tensor_tensor(s, sums[0], sums[1], AL.add)
                sums = sums[2:] + [s]
            pe_rhs = [prods[k] for k in PE_ACC_TAPS] + sums
            pagg = poolB.tile([C, NPX], F32, tag="pB")
            n = len(pe_rhs)
            for i, r in enumerate(pe_rhs):
                for h in range(2):
                    cs = slice(512 * h, 512 * h + 512)
                    nc.tensor.matmul(pagg[:, cs], lhsT=id_sb, rhs=r[:, cs],
                                     start=(i == 0), stop=(i == n - 1))
            av = agg_slab[:, t * NPX:(t + 1) * NPX]
            nc.scalar.activation(av, pagg, AF.Relu,
                                 accum_out=slots_a[:, t:t + 1])

            if t == SPLIT_T - 1:
                g2A = emit_se("A", 0, SPLIT_T, cc_inA, cc_outA)

        # ---------------- SE attention tail ----------------
        g2B = emit_se("B", SPLIT_T, NT, cc_inB, cc_outB)
        gap2 = smallp.tile([C, 1], F32, tag="gapT")
        nc.vector.tensor_tensor(gap2, g2A, g2B, AL.add)

        ph = poolA.tile([64, 1], F32, tag="pA")
        nc.tensor.matmul(ph, lhsT=ws1_sb, rhs=gap2, start=True, stop=True)
        hso = smallp.tile([64, 1], F32, tag="h")
        nc.scalar.activation(hso, ph, AF.Relu, bias=bs1_sb[:, 0:1])
        pa = poolA.tile([C, 2], F32, tag="pA")
        nc.tensor.matmul(pa[:, 0:1], lhsT=ws2_sb[:, 0, :], rhs=hso,
                         start=True, stop=True)
        nc.tensor.matmul(pa[:, 1:2], lhsT=ws2_sb[:, 1, :], rhs=hso,
                         start=True, stop=True)
        a01 = smallp.tile([C, 2], F32, tag="a01")
        nc.scalar.activation(a01[:, 0:1], pa[:, 0:1], AF.Identity,
                             bias=bs2_sb[:, 0:1])
        nc.scalar.activation(a01[:, 1:2], pa[:, 1:2], AF.Identity,
                             bias=bs2_sb[:, 1:2])
        dse = smallp.tile([C, 1], F32, tag="dse")
        nc.vector.tensor_tensor(dse, a01[:, 0:1], a01[:, 1:2], AL.subtract)
        nc.scalar.activation(attn_sb[:, 0:1], dse, AF.Sigmoid)
        nc.scalar.activation(attn_sb[:, 1:2], dse, AF.Sigmoid, scale=-1.0)
        nc.vector.tensor_scalar(diag0_sb, id_sb, attn_sb[:, 0:1], None, AL.mult)
        nc.vector.tensor_scalar(diag1_sb, id_sb, attn_sb[:, 1:2], None, AL.mult)

        # ---------------- phase 2: diag-matmul blend + store ----------------
        for t in range(NT):
            kv = kemb_slab[:, t * NPX:(t + 1) * NPX]
            av = agg_slab[:, t * NPX:(t + 1) * NPX]
            p2 = poolA.tile([C, NPX], F32, tag="pA")
            nc.tensor.matmul(p2[:, 0:512], lhsT=diag0_sb, rhs=av[:, 0:512],
                             start=True, stop=False)
            nc.tensor.matmul(p2[:, 512:1024], lhsT=diag0_sb, rhs=av[:, 512:1024],
                             start=True, stop=False)
            nc.tensor.matmul(p2[:, 0:512], lhsT=diag1_sb, rhs=kv[:, 0:512],
                             start=False, stop=True)
            nc.tensor.matmul(p2[:, 512:1024], lhsT=diag1_sb, rhs=kv[:, 512:1024],
                             start=False, stop=True)
            outf = outp.tile([C, NPX], F32, tag="outf")
            if P2_DVE_MOD and (t % P2_DVE_MOD == 0):
                nc.vector.tensor_scalar(outf, p2, 0.0, None, AL.add)
            else:
                nc.scalar.activation(outf, p2, AF.Copy)
            nc.sync.dma_start(out_d.ap()[:, t * NPX:(t + 1) * NPX], outf)

    return nc


_CACHE = {}


def _get_nc():
    if "nc" not in _CACHE:
        nc = bacc.Bacc("TRN2", target_bir_lowering=False, debug=False,
                       num_devices=NCORES)
        _build_kernel(nc)
        nc.compile()
        _CACHE["nc"] = nc
    return _CACHE["nc"]


def make_in_maps(inputs):
    x = np.asarray(inputs["x"], np.float32)
    wts = _prep_weights(inputs)
    xp = np.pad(x, ((0, 0), (0, 0), (1, 1), (1, 1))).astype(BF)
    in_maps = []
    for core in range(NCORES):
        bb, q = divmod(core, 4)
        slab = np.ascontiguousarray(xp[bb, :, RQ * q:RQ * q + RQ + 2, :])
        m = {"xs": slab}
        m.update(wts)
        in_maps.append(m)
    return in_maps


def kernel(**inputs):
    in_maps = make_in_maps(inputs)
    nc = _get_nc()
    res = run_bass_kernel_spmd(nc, in_maps, core_ids=list(range(NCORES)))
    out = np.empty((B, C, H, W), np.float32)
    for core in range(NCORES):
        bb, q = divmod(core, 4)
        out[bb, :, RQ * q:RQ * q + RQ, :] = \
            res.results[core]["out"].reshape(C, RQ, W)
    return out
